# revision 9
# baseline (speedup 1.0000x reference)
"""Batch Graph-Attention layer (GAT, eval mode) on 8 Trainium2 NeuronCores. v2

Math per graph b (reference):
    Wh = h @ W                         (N=1024, Fo=64)
    f1 = Wh @ a1 ; f2 = Wh @ a2        (N,)
    e[i,j]   = leakyrelu(f1[i]+f2[j], 0.01)
    att      = softmax(e, axis=j)
    out      = elu(att @ Wh)

Device algorithm (per graph):
    exp(lrelu(x)) == max(exp(x), exp(0.01x))   (exact for slope in (0,1))
    expe[i,j] = g1[i] * hh2[j] * v[j,i],  v[j,i] = max(q[j], r1[i])
      q[j]  = exp(0.99 f2[j]),  r1[i] = exp(-0.99 f1[i]),  hh2 = exp(0.01 f2)
    g1 cancels in the softmax ratio.  hh2 is absorbed into the weights:
      whb[j, 0:64] = hh2[j]*Wh[j,:],  whb[j, 64] = hh2[j]
    numer.T[o,i] & rowsum[i] via PE:  lhsT = whb (65 cols), rhs = v
    out[i,:] = elu(numer[i,:]/rowsum[i]),
    elu(x) = max(x, min(exp(x)-1, 0))   (exact identity)

v2 structural changes vs v1:
  - u phase is a single-op TS max (hh2 folded into whb) instead of mult+max.
  - whb built by one broadcast tensor_tensor multiply per half (no memset).
  - per-purpose PSUM pools (no cross-stage buffer entanglement).
  - h loaded as two 2KB-per-partition DMAs per graph (one DIRECT2D each).
  - B matmuls half-serial so each graph's first C half starts early.

Sharding: batch dim 16 -> 8 cores x 2 graphs (pure data parallel).
Node ids use the (p c) permutation; store applies the inverse permutation.
"""

import numpy as np

import concourse.bass as bass
import concourse.mybir as mybir
import concourse.tile as tile
from concourse import bacc

F32 = mybir.dt.float32
BF16 = mybir.dt.bfloat16
AF = mybir.ActivationFunctionType
OP = mybir.AluOpType

B_PER_CORE = 2
N = 1024
F_IN = 128
F_OUT = 64
C = N // 128  # 8 chunks of 128 rows
NEG_SLOPE = 0.01
WARM = 13

LAST_PERF = {}


def build_bass():
    nc = bacc.Bacc("TRN2", target_bir_lowering=False, debug=False)

    h_d = nc.dram_tensor("h", [B_PER_CORE, N, F_IN], F32, kind="ExternalInput")
    w_d = nc.dram_tensor("W", [B_PER_CORE, F_IN, F_OUT], F32, kind="ExternalInput")
    a_d = nc.dram_tensor("a", [B_PER_CORE, 1, 2 * F_OUT, 1], F32, kind="ExternalInput")
    o_d = nc.dram_tensor("out", [B_PER_CORE, N, F_OUT], F32, kind="ExternalOutput")

    with tile.TileContext(nc) as tc:
        with (
            tc.tile_pool(name="singles", bufs=1) as singles,
            tc.tile_pool(name="hin", bufs=2) as hin_pool,
            tc.tile_pool(name="ht", bufs=2) as ht_pool,
            tc.tile_pool(name="small", bufs=2) as small_pool,
            tc.tile_pool(name="bcast", bufs=2) as bcast_pool,
            tc.tile_pool(name="v", bufs=16) as v_pool,
            tc.tile_pool(name="tail", bufs=2) as tail_pool,
            tc.tile_pool(name="ps_wt", bufs=1, space="PSUM") as ps_wt,
            tc.tile_pool(name="ps_pt", bufs=2, space="PSUM") as ps_pt,
            tc.tile_pool(name="ps_pwh", bufs=2, space="PSUM") as ps_pwh,
            tc.tile_pool(name="ps_f1b", bufs=2, space="PSUM") as ps_f1b,
            tc.tile_pool(name="ps_php", bufs=1, space="PSUM") as ps_php,
        ):
            # identity built on-chip (no DMA): 1 where col==row else 0
            ident = singles.tile([128, 128], F32)
            nc.gpsimd.memset(ident[:], 1.0)
            nc.gpsimd.affine_select(
                ident[:], ident[:], pattern=[[1, 128]],
                compare_op=OP.is_equal, fill=0.0,
                base=0, channel_multiplier=-1,
            )
            identb = singles.tile([128, 128], BF16)
            nc.gpsimd.tensor_copy(identb[:], ident[:])
            zeros = singles.tile([128, 128], F32)
            nc.vector.memset(zeros[:], 0.0)
            zeros_wide = singles.tile([128, 4 * F_OUT], F32)
            nc.gpsimd.memset(zeros_wide[:], 0.0)

            warm_ps = ps_wt.tile([128, 128], F32, tag="pswt")

            st = [dict() for _ in range(B_PER_CORE)]

            def stage_load_w(b):
                """W first on its ring (tiny) so W-prep PE work starts early."""
                s = st[b]
                dmaw = nc.scalar if b == 0 else nc.sync
                wext = small_pool.tile([128, F_OUT + 1], F32, tag="wext")
                dmaw.dma_start(out=wext[:, 0:F_OUT], in_=w_d[b])
                s.update(wext=wext)

            def stage_load_a(b):
                s = st[b]
                dmaw = nc.scalar if b == 0 else nc.sync
                apair = small_pool.tile([F_OUT, 2], F32, tag="apair")
                dmaw.dma_start(
                    out=apair[:],
                    in_=a_d[b, 0, :, 0].rearrange("(two o) -> o two", two=2),
                )
                s.update(apair=apair)

            def stage_h(b):
                """One dma_start per graph: per partition the (c f) source is
                4KB contiguous in HBM, so issue cost is one DIRECT2D."""
                s = st[b]
                h_sb = hin_pool.tile([128, C * F_IN], F32, tag="h")
                h_src = h_d[b].rearrange("(p c) f -> p (c f)", c=C)
                eng = nc.sync if b == 0 else nc.scalar
                for half in range(2):
                    eng.dma_start(
                        out=h_sb[:, half * 512 : (half + 1) * 512],
                        in_=h_src[:, half * 512 : (half + 1) * 512],
                    )
                s["h_sb"] = h_sb

            def stage_wt(b):
                """W.T, wa12 = W @ [a1|a2], wa1 broadcast, bf16 W-ext."""
                s = st[b]
                wext = s["wext"]
                wt_ps = ps_wt.tile([F_OUT, 128], F32, tag="pswt")
                nc.tensor.transpose(wt_ps[:], wext[:, 0:F_OUT], ident[:])
                wt_sb = small_pool.tile([F_OUT, 128], F32, tag="wt")
                nc.vector.tensor_copy(wt_sb[:], wt_ps[:])

                wa_ps = ps_wt.tile([128, 2], F32, tag="pswt")
                nc.tensor.matmul(wa_ps[:], wt_sb[:], s["apair"][:])
                wa_sb = small_pool.tile([128, 2], F32, tag="wa")
                nc.vector.tensor_copy(wa_sb[:], wa_ps[:])
                # wa2 becomes column 64 of the Wh matmul rhs -> f2 per chunk
                nc.vector.tensor_copy(wext[:, F_OUT : F_OUT + 1], wa_sb[:, 1:2])
                wextb = small_pool.tile([128, F_OUT + 1], BF16, tag="wextb")
                nc.vector.tensor_copy(wextb[:], wext[:])
                # broadcast wa1 along free dim -> lhsT for the f1-broadcast mm
                wa1b = small_pool.tile([128, 128], BF16, tag="wa1b")
                nc.vector.tensor_scalar(
                    wa1b[:], zeros[:], wa_sb[:, 0:1], None, op0=OP.add
                )
                s.update(wextb=wextb, wa1b=wa1b)

            def stage_ht_half(b, half):
                """PE f32 transposes of h chunks; PSUM->SBUF copy casts to
                bf16 (ScalarE for half0, DVE for half1)."""
                s = st[b]
                if half == 0:
                    ht_sb = ht_pool.tile([128, C, 128], BF16, tag="ht")
                    s["ht"] = ht_sb
                ht_sb = s["ht"]
                pt = ps_pt.tile([128, 4, 128], F32, tag="pspt", name=f"pt{b}_{half}")
                for qq in range(4):
                    c = half * 4 + qq
                    nc.tensor.transpose(
                        pt[:, qq, :],
                        s["h_sb"][:, c * F_IN : (c + 1) * F_IN],
                        ident[:],
                    )
                dst = ht_sb[:, half * 4 : (half + 1) * 4, :]
                if half == 0:
                    nc.scalar.copy(dst, pt[:])
                else:
                    nc.vector.tensor_copy(dst, pt[:])

            def stage_a_half(b, half):
                """One half of phase A: f1b MM + r1b exp, Wh MMs + q/hh2 exps,
                whb build."""
                s = st[b]
                ht_sb = s["ht"]
                if half == 0:
                    s["pwh"] = [None, None]
                    s["qs"] = small_pool.tile([128, C], F32, tag="qs", name=f"qs{b}")
                    s["hh2"] = small_pool.tile([128, C], F32, tag="hh2", name=f"hh2{b}")
                    s["r1b"] = bcast_pool.tile([128, N], BF16, tag="r1b", name=f"r1b{b}")
                    s["whb"] = small_pool.tile(
                        [128, C, F_OUT + 2], BF16, tag="whb", name=f"whb{b}"
                    )
                qs, hh2, r1b, whb = s["qs"], s["hh2"], s["r1b"], s["whb"]
                sl = slice(half * 4, (half + 1) * 4)

                # f1 broadcast to all 128 partitions via PE (bf16 fast path)
                pf1b = ps_f1b.tile([128, 512], F32, tag="psf1b", name=f"pf1b{b}_{half}")
                hsrc = ht_sb[:, sl, :].rearrange("p c f -> p (c f)")
                nc.tensor.matmul(pf1b[:], s["wa1b"][:], hsrc)

                # Wh chunks (+f2 col)
                pwh = ps_pwh.tile([128, 4, F_OUT + 1], F32, tag="pspwh", name=f"pwh{b}_{half}")
                s["pwh"][half] = pwh
                for qq in range(4):
                    c = half * 4 + qq
                    nc.tensor.matmul(pwh[:, qq, :], ht_sb[:, c, :], s["wextb"][:])

                # r1b = exp(-0.99 f1b) (ACT, bf16 out)
                nc.scalar.activation(
                    r1b[:, half * 512 : (half + 1) * 512], pf1b[:], AF.Exp,
                    scale=-(1.0 - NEG_SLOPE),
                )
                # qs = exp(0.99 f2), hh2 = exp(0.01 f2)
                nc.scalar.activation(
                    qs[:, sl], pwh[:, :, F_OUT], AF.Exp, scale=(1.0 - NEG_SLOPE)
                )
                nc.scalar.activation(
                    hh2[:, sl], pwh[:, :, F_OUT], AF.Exp, scale=NEG_SLOPE
                )

                # whb = [hh2*Wh | hh2 | pad]
                hh23 = hh2[:].rearrange("p (c one) -> p c one", one=1)
                b_wh, b_hh2 = bass.broadcast_tensor_aps(
                    pwh[:, :, 0:F_OUT], hh23[:, sl, :]
                )
                nc.vector.tensor_tensor(whb[:, sl, 0:F_OUT], b_wh, b_hh2, op=OP.mult)
                nc.vector.tensor_copy(
                    whb[:, sl, F_OUT : F_OUT + 1], hh23[:, sl, :]
                )

            def stage_b(b):
                """Per chunk: v_c = max(r1b, q_c) (single-op TS), then the two
                accumulating half matmuls for that chunk."""
                s = st[b]
                if b == 0:
                    for _ in range(3):
                        nc.tensor.matmul(warm_ps[:], zeros[:], zeros[:])
                php = [
                    ps_php.tile([F_OUT + 1, 512], F32, tag="psphp", name=f"phpT{b}_{h}")
                    for h in range(2)
                ]
                s["phpT"] = {0: php[0], 1: php[1]}
                s["v"] = []
                for c in range(C):
                    v = v_pool.tile([128, N], BF16, tag="v", name=f"v{b}_{c}")
                    s["v"].append(v)
                    nc.vector.tensor_scalar(
                        v[:], s["r1b"][:], s["qs"][:, c : c + 1], None, op0=OP.max
                    )
                    nc.tensor.matmul(
                        php[0][:],
                        s["whb"][:, c, 0 : F_OUT + 1],
                        v[:, 0:512],
                        start=(c == 0),
                        stop=(c == C - 1),
                    )

            def stage_b_h1(b):
                s = st[b]
                php = s["phpT"]
                for c in range(C):
                    nc.tensor.matmul(
                        php[1][:],
                        s["whb"][:, c, 0 : F_OUT + 1],
                        s["v"][c][:, 512:1024],
                        start=(c == 0),
                        stop=(c == C - 1),
                    )

            def stage_c_half(b, half):
                """One half: copy, transpose-back, normalize+ELU, store."""
                s = st[b]
                if half == 0:
                    hpT_t = tail_pool.tile([F_OUT + 1, N], BF16, tag="hpT")
                    rz_t = small_pool.tile([128, C], F32, tag="rz")
                    hp_t = tail_pool.tile([128, C, F_OUT], F32, tag="hp")
                    te_t = tail_pool.tile([128, C, F_OUT], F32, tag="te")
                    sm_t = tail_pool.tile([128, C, F_OUT], F32, tag="sm")
                    osb_t = tail_pool.tile([128, C, F_OUT], F32, tag="osb")
                    s.update(hpT=hpT_t, rz=rz_t, hp=hp_t, te=te_t, sm=sm_t,
                             osb=osb_t)
                hpT_sb, rz = s["hpT"], s["rz"]
                te, sm, osb = s["te"], s["sm"], s["osb"]
                dst = hpT_sb[:, half * 512 : (half + 1) * 512]
                if half == 0:
                    nc.scalar.copy(dst, s["phpT"][half][:])
                else:
                    nc.vector.tensor_copy(dst, s["phpT"][half][:])
                php = ps_pt.tile([128, 4, F_OUT + 2], BF16, tag="pspt", name=f"php{b}_{half}")
                for qq in range(4):
                    c = half * 4 + qq
                    nc.tensor.transpose(
                        php[:, qq, 0 : F_OUT + 1],
                        hpT_sb[:, c * 128 : (c + 1) * 128],
                        identb[: F_OUT + 1, : F_OUT + 1],
                    )
                sl = slice(half * 4, (half + 1) * 4)
                nc.vector.reciprocal(rz[:, sl], php[:, :, F_OUT])
                rz3 = rz[:, sl].rearrange("p (c one) -> p c one", one=1)
                b_in0, b_rz = bass.broadcast_tensor_aps(php[:, :, 0:F_OUT], rz3)
                hp = s["hp"]
                nc.vector.tensor_tensor(hp[:, sl, :], b_in0, b_rz, op=OP.mult)
                nc.scalar.activation(te[:, sl, :], hp[:, sl, :], AF.Exp)
                nc.vector.tensor_scalar(
                    sm[:, sl, :], te[:, sl, :], 1.0, 0.0,
                    op0=OP.subtract, op1=OP.min,
                )
                nc.vector.tensor_tensor(
                    osb[:, sl, :], hp[:, sl, :], sm[:, sl, :], op=OP.max
                )
                o_dst = o_d[b].rearrange("(p c) o -> p (c o)", c=C)
                osb_flat = osb[:].rearrange("p c o -> p (c o)")
                w = 4 * F_OUT
                eng = (nc.sync, nc.scalar) if b == 0 else (nc.scalar, nc.sync)
                eng[half].dma_start(
                    out=o_dst[:, half * w : (half + 1) * w],
                    in_=osb_flat[:, half * w : (half + 1) * w],
                )

            # emission order == ring issue order for DMAs; PE work interleaved
            stage_load_w(0)
            stage_h(0)
            stage_load_a(0)
            stage_load_w(1)
            stage_h(1)
            stage_load_a(1)
            # PE warm-up FIRST in PE queue order: >=3.4us of sustained junk
            # matmuls so the HAM clock gate reaches K=8/8 before real work
            for _ in range(WARM):
                nc.tensor.matmul(warm_ps[:], zeros[:], zeros[:])
            stage_wt(0)
            stage_wt(1)
            stage_ht_half(0, 0)
            stage_ht_half(0, 1)
            stage_a_half(0, 0)
            stage_ht_half(1, 0)
            stage_a_half(0, 1)
            stage_ht_half(1, 1)
            stage_a_half(1, 0)
            stage_a_half(1, 1)
            stage_b(0)
            stage_c_half(0, 0)
            stage_b_h1(0)
            stage_b(1)
            stage_c_half(0, 1)
            stage_b_h1(1)
            stage_c_half(1, 0)
            stage_c_half(1, 1)

    nc.compile()
    return nc


def kernel(h: np.ndarray, W: np.ndarray, a: np.ndarray, _trace: bool = False):
    from concourse.bass_utils import run_bass_kernel_spmd

    n_cores = 8
    nc = build_bass()
    in_maps = []
    for i in range(n_cores):
        sl = slice(i * B_PER_CORE, (i + 1) * B_PER_CORE)
        in_maps.append(
            {
                "h": np.ascontiguousarray(h[sl]),
                "W": np.ascontiguousarray(W[sl]),
                "a": np.ascontiguousarray(a[sl]),
            }
        )
    res = run_bass_kernel_spmd(
        nc, in_maps, core_ids=list(range(n_cores)), trace=_trace
    )
    LAST_PERF.clear()
    LAST_PERF.update(
        {
            "exec_time_ns": res.exec_time_ns,
            "mean_exec_time_ns": res.mean_exec_time_ns,
            "trace": res.instructions_and_trace[1]
            if res.instructions_and_trace
            else None,
        }
    )
    return np.concatenate([r["out"] for r in res.results], axis=0)
